# revision 7
# baseline (speedup 1.0000x reference)
"""Multi-head self-attention on 8 Trainium2 NeuronCores.

Tensor-parallel over heads: core c owns heads 2c, 2c+1 (128 of the 1024
hidden columns).  Each core:
  1. Qt/Kt = (x @ wq/wk + b)^T  in [d, token] layout (d on partitions,
     2 heads stacked: partitions 0:64 head0, 64:128 head1)
  2. V_aug = x @ [wv | 0] + [bv | 1]  in [token, 65-per-head] layout
     (ones column -> softmax denominator comes out of the P@V matmul)
  3. per (batch, head): S^T tiles = K^T.T @ Q^T  (contraction d=64),
     P^T = exp(S^T / 8) (no max subtraction needed: |S/8| < ~3),
     out^T[65, s] = V_aug.T @ P^T accumulated over t tiles,
     attnT = out^T[0:64] * broadcast(1 / out^T[64])
  4. partial = attnT.T @ wo[128 rows of this core]  -> HBM (f32)
Host sums the 8 partials and adds bo.

Shapes hardcoded for x:[2,2048,1024], 16 heads, d_k=64.
"""

import numpy as np
import ml_dtypes

import concourse.bass as bass
import concourse.tile as tile
from concourse import bacc, mybir
from concourse.bass import ts
from concourse.bass_utils import run_bass_kernel_spmd

BF16 = mybir.dt.bfloat16
F32 = mybir.dt.float32
NPBF16 = ml_dtypes.bfloat16

B = 2
S = 2048
D = 1024
NT = B * S  # 4096 tokens
DK = 64
NCORES = 8
HPC = 2  # heads per core
SC = 1024  # attention s-chunk (exp op free size)

_CACHE = {}


def _build_nc():
    nc = bacc.Bacc("TRN2", target_bir_lowering=False, debug=False,
                   num_devices=NCORES)

    xT = nc.dram_tensor("xT", [D, NT], BF16, kind="ExternalInput").ap()
    wq = nc.dram_tensor("wq", [D, 128], BF16, kind="ExternalInput").ap()
    wk = nc.dram_tensor("wk", [D, 128], BF16, kind="ExternalInput").ap()
    wv = nc.dram_tensor("wv", [D, 130], BF16, kind="ExternalInput").ap()
    bq = nc.dram_tensor("bq", [128, 1], F32, kind="ExternalInput").ap()
    bk = nc.dram_tensor("bk", [128, 1], F32, kind="ExternalInput").ap()
    bv = nc.dram_tensor("bv", [128, 130], F32, kind="ExternalInput").ap()
    wo = nc.dram_tensor("wo", [128, D], BF16, kind="ExternalInput").ap()
    out = nc.dram_tensor("out", [NT, D], F32, kind="ExternalOutput").ap()

    with tile.TileContext(nc) as tc:
        _emit(nc, tc, xT, wq, wk, wv, bq, bk, bv, wo, out)
    nc.compile()
    return nc


def _emit(nc, tc, xT, wq, wk, wv, bq, bk, bv, wo, out):
    import contextlib
    ctx = contextlib.ExitStack()
    with ctx:
        consts = ctx.enter_context(tc.tile_pool(name="consts", bufs=1))
        ptp = ctx.enter_context(tc.tile_pool(name="ptp", bufs=34))
        psp = ctx.enter_context(tc.tile_pool(name="psp", bufs=3, space="PSUM"))
        psb = ctx.enter_context(tc.tile_pool(name="psb", bufs=2, space="PSUM"))
        stg = ctx.enter_context(tc.tile_pool(name="stg", bufs=4))
        nrm = ctx.enter_context(tc.tile_pool(name="nrm", bufs=2))

        # ---- persistent SBUF tensors ----
        xT_sb = consts.tile([128, 8, NT], BF16)      # 8 k-tiles of x^T
        wq_sb = consts.tile([128, 8, 128], BF16)
        wk_sb = consts.tile([128, 8, 128], BF16)
        wv_sb = consts.tile([128, 8, 130], BF16)
        bq_sb = consts.tile([128, 1], F32)
        bk_sb = consts.tile([128, 1], F32)
        bv_sb = consts.tile([128, 130], F32)
        wo_sb = consts.tile([128, D], BF16)
        QT = consts.tile([128, NT], BF16)
        KT = consts.tile([128, NT], BF16)
        V_sb = consts.tile([128, 32, 130], BF16)     # [t-in-tile, t_tile, col]
        attnT = consts.tile([128, NT], BF16)

        xT_d = xT.rearrange("(k p) n -> k p n", p=128)
        wq_d = wq.rearrange("(k p) c -> k p c", p=128)
        wk_d = wk.rearrange("(k p) c -> k p c", p=128)
        wv_d = wv.rearrange("(k p) c -> k p c", p=128)
        for k in range(8):
            nc.sync.dma_start(out=xT_sb[:, k, :], in_=xT_d[k])
            nc.sync.dma_start(out=wq_sb[:, k, :], in_=wq_d[k])
            nc.sync.dma_start(out=wk_sb[:, k, :], in_=wk_d[k])
            nc.sync.dma_start(out=wv_sb[:, k, :], in_=wv_d[k])
        nc.sync.dma_start(out=bq_sb, in_=bq)
        nc.sync.dma_start(out=bk_sb, in_=bk)
        nc.sync.dma_start(out=bv_sb, in_=bv)
        nc.sync.dma_start(out=wo_sb, in_=wo)

        # ---- emit helpers for PE work that can fill ACT-bound phases ----
        def emit_v_tile(tt):
            # V_aug [t,130] += xT[k, t-tile].T @ wv[k]
            psv = psb.tile([128, 512], F32, tag="psb")
            for k in range(8):
                nc.tensor.matmul(psv[:, 0:130], lhsT=xT_sb[:, k, ts(tt, 128)],
                                 rhs=wv_sb[:, k, :],
                                 start=(k == 0), stop=(k == 7))
            nc.vector.tensor_add(V_sb[:, tt, :], psv[:, 0:130], bv_sb)

        def emit_wo_tile(tt):
            for eh in range(2):
                pw = psb.tile([128, 512], F32, tag="psb")
                nc.tensor.matmul(pw, lhsT=attnT[:, ts(tt, 128)],
                                 rhs=wo_sb[:, ts(eh, 512)],
                                 start=True, stop=True)
                ob = stg.tile([128, 512], F32, tag="ob")
                nc.vector.tensor_copy(ob, pw)
                nc.sync.dma_start(
                    out=out[tt * 128:(tt + 1) * 128, eh * 512:(eh + 1) * 512],
                    in_=ob)

        # ---- Q^T / K^T projections: [c,128] += w[k].T @ xT[k] ----
        for w_sb, b_sb, o_sb in ((wq_sb, bq_sb, QT), (wk_sb, bk_sb, KT)):
            for n in range(8):  # 512-token chunks
                ps = psb.tile([128, 512], F32, tag="psb")
                for k in range(8):
                    nc.tensor.matmul(ps, lhsT=w_sb[:, k, :],
                                     rhs=xT_sb[:, k, ts(n, 512)],
                                     start=(k == 0), stop=(k == 7))
                nc.vector.tensor_scalar_add(o_sb[:, ts(n, 512)], ps, b_sb)

        # ---- V_aug projection for batch 0 ----
        for tt in range(16):
            emit_v_tile(tt)

        # ---- attention + output projection ----
        # "extras": independent PE work interleaved into ACT-bound phases
        for b in range(B):
            for sc in range(S // SC):
                if b == 0:
                    extras = [lambda t=t: emit_v_tile(t)
                              for t in range(16 + sc * 8, 16 + (sc + 1) * 8)]
                else:
                    extras = [lambda t=t: emit_wo_tile(t)
                              for t in range(sc * 8, (sc + 1) * 8)]
                s0 = b * S + sc * SC
                pts = []
                for tt in range(16):
                    row = []
                    for h in range(HPC):
                        ps = psp.tile([128, SC], F32, tag="ps")
                        hsl = slice(h * DK, (h + 1) * DK)
                        for n2 in range(SC // 512):
                            nc.tensor.matmul(
                                ps[:, ts(n2, 512)],
                                lhsT=KT[hsl, b * S + tt * 128:b * S + (tt + 1) * 128],
                                rhs=QT[hsl, s0 + n2 * 512:s0 + (n2 + 1) * 512],
                                start=True, stop=True)
                        pt = ptp.tile([128, SC], BF16, tag="pt")
                        nc.scalar.activation(
                            out=pt, in_=ps,
                            func=mybir.ActivationFunctionType.Exp,
                            scale=0.125)
                        row.append(pt)
                    pts.append(row)
                    if tt % 2 == 1 and extras:
                        extras.pop(0)()
                for h in range(HPC):
                    pso = psp.tile([128, SC], F32, tag="ps")
                    for tt in range(16):
                        for n2 in range(SC // 512):
                            nc.tensor.matmul(
                                pso[0:65, ts(n2, 512)],
                                lhsT=V_sb[:, b * 16 + tt, h * 65:(h + 1) * 65],
                                rhs=pts[tt][h][:, ts(n2, 512)],
                                start=(tt == 0), stop=(tt == 15))
                    rsum = nrm.tile([1, SC], F32, tag="rsum")
                    nc.vector.tensor_copy(rsum, pso[64:65, :])
                    rec = nrm.tile([1, SC], F32, tag="rec")
                    nc.vector.reciprocal_approx_fast(out=rec, in_=rsum)
                    recb = nrm.tile([64, SC], F32, tag="recb")
                    nc.gpsimd.partition_broadcast(recb, rec)
                    nc.vector.tensor_mul(
                        attnT[h * DK:(h + 1) * DK, s0:s0 + SC],
                        pso[0:64, :], recb)
                for e in extras:
                    e()
        # remaining output projection (batch 1 tokens)
        for tt in range(16, 32):
            emit_wo_tile(tt)


def _prep_in_maps(x, wq, bq, wk, bk, wv, bv, wo):
    x2 = np.asarray(x, np.float32).reshape(NT, D)
    xT = np.ascontiguousarray(x2.T).astype(NPBF16)
    wq = np.asarray(wq, np.float32)
    wk = np.asarray(wk, np.float32)
    wv = np.asarray(wv, np.float32)
    wo = np.asarray(wo, np.float32)
    bq = np.asarray(bq, np.float32)
    bk = np.asarray(bk, np.float32)
    bv = np.asarray(bv, np.float32)
    in_maps = []
    for c in range(NCORES):
        cs = slice(c * 128, (c + 1) * 128)
        wv_aug = np.zeros((D, 130), np.float32)
        wv_aug[:, 0:64] = wv[:, c * 128:c * 128 + 64]
        wv_aug[:, 65:129] = wv[:, c * 128 + 64:c * 128 + 128]
        bv_aug = np.zeros(130, np.float32)
        bv_aug[0:64] = bv[c * 128:c * 128 + 64]
        bv_aug[64] = 1.0
        bv_aug[65:129] = bv[c * 128 + 64:c * 128 + 128]
        bv_aug[129] = 1.0
        in_maps.append({
            "xT": xT,
            "wq": wq[:, cs].astype(NPBF16),
            "wk": wk[:, cs].astype(NPBF16),
            "wv": wv_aug.astype(NPBF16),
            "bq": np.ascontiguousarray(bq[cs].reshape(128, 1)),
            "bk": np.ascontiguousarray(bk[cs].reshape(128, 1)),
            "bv": np.ascontiguousarray(np.broadcast_to(bv_aug, (128, 130))),
            "wo": wo[cs, :].astype(NPBF16),
        })
    return in_maps


def kernel(x, wq, bq, wk, bk, wv, bv, wo, bo, _run_kwargs=None):
    if "nc" not in _CACHE:
        _CACHE["nc"] = _build_nc()
    nc = _CACHE["nc"]
    in_maps = _prep_in_maps(x, wq, bq, wk, bk, wv, bv, wo)
    res = run_bass_kernel_spmd(nc, in_maps, list(range(NCORES)),
                               **(_run_kwargs or {}))
    acc = np.zeros((NT, D), np.float32)
    for c in range(NCORES):
        acc += res.results[c]["out"]
    acc += np.asarray(bo, np.float32)[None, :]
    if _run_kwargs:
        _CACHE["last_results"] = res
    return acc.reshape(B, S, D)


# revision 11
# speedup vs baseline: 1.2462x; 1.2462x over previous
"""Multi-head self-attention on 8 Trainium2 NeuronCores.

Tensor-parallel over heads: core c owns heads 2c, 2c+1 (128 of the 1024
hidden columns).  Each core:
  1. Qt/Kt = (x @ wq/wk + b)^T  in [d, token] layout (d on partitions,
     2 heads stacked: partitions 0:64 head0, 64:128 head1)
  2. V_aug = x @ [wv | 0] + [bv | 1]  in [token, 65-per-head] layout
     (ones column -> softmax denominator comes out of the P@V matmul)
  3. per (batch, head): S^T tiles = K^T.T @ Q^T  (contraction d=64),
     P^T = exp(S^T / 8) (no max subtraction needed: |S/8| < ~3),
     out^T[65, s] = V_aug.T @ P^T accumulated over t tiles,
     attnT = out^T[0:64] * broadcast(1 / out^T[64])
  4. partial = attnT.T @ wo[128 rows of this core]  -> HBM (f32)
Host sums the 8 partials and adds bo.

Shapes hardcoded for x:[2,2048,1024], 16 heads, d_k=64.
"""

import numpy as np
import ml_dtypes

import concourse.bass as bass
import concourse.tile as tile
from concourse import bacc, mybir
from concourse.bass import ts
from concourse.bass_utils import run_bass_kernel_spmd

BF16 = mybir.dt.bfloat16
F32 = mybir.dt.float32
NPBF16 = ml_dtypes.bfloat16

B = 2
S = 2048
D = 1024
NT = B * S  # 4096 tokens
DK = 64
NCORES = 8
HPC = 2  # heads per core
SC = 1024  # attention s-chunk (exp op free size)

_CACHE = {}


def _build_nc():
    nc = bacc.Bacc("TRN2", target_bir_lowering=False, debug=False,
                   num_devices=NCORES)

    xT = nc.dram_tensor("xT", [D, NT], BF16, kind="ExternalInput").ap()
    wq = nc.dram_tensor("wq", [D, 128], BF16, kind="ExternalInput").ap()
    wk = nc.dram_tensor("wk", [D, 128], BF16, kind="ExternalInput").ap()
    wv = nc.dram_tensor("wv", [D, 130], BF16, kind="ExternalInput").ap()
    bq = nc.dram_tensor("bq", [128, 1], F32, kind="ExternalInput").ap()
    bk = nc.dram_tensor("bk", [128, 1], F32, kind="ExternalInput").ap()
    bv = nc.dram_tensor("bv", [128, 130], F32, kind="ExternalInput").ap()
    wo = nc.dram_tensor("wo", [128, D], BF16, kind="ExternalInput").ap()
    out = nc.dram_tensor("out", [NT, D], F32, kind="ExternalOutput").ap()

    with tile.TileContext(nc) as tc:
        _emit(nc, tc, xT, wq, wk, wv, bq, bk, bv, wo, out)
    nc.compile()
    return nc


def _emit(nc, tc, xT, wq, wk, wv, bq, bk, bv, wo, out):
    import contextlib
    ctx = contextlib.ExitStack()
    with ctx:
        consts = ctx.enter_context(tc.tile_pool(name="consts", bufs=1))
        ptp = ctx.enter_context(tc.tile_pool(name="ptp", bufs=36))
        psp = ctx.enter_context(tc.tile_pool(name="psp", bufs=2, space="PSUM"))
        pvp = ctx.enter_context(tc.tile_pool(name="pvp", bufs=2, space="PSUM"))
        stg = ctx.enter_context(tc.tile_pool(name="stg", bufs=4))
        nrm = ctx.enter_context(tc.tile_pool(name="nrm", bufs=2))

        # ---- persistent SBUF tensors ----
        xT_sb = consts.tile([128, 8, NT], BF16)      # 8 k-tiles of x^T
        wq_sb = consts.tile([128, 8, 128], BF16)
        wk_sb = consts.tile([128, 8, 128], BF16)
        wv_sb = consts.tile([128, 8, 130], BF16)
        bq_sb = consts.tile([128, 1], F32)
        bk_sb = consts.tile([128, 1], F32)
        bv_sb = consts.tile([128, 130], F32)
        wo_sb = consts.tile([128, D], BF16)
        QT = consts.tile([128, NT], BF16)
        KT = consts.tile([128, NT], BF16)
        V_sb = consts.tile([128, 32, 130], BF16)     # [t-in-tile, t_tile, col]
        attnT = consts.tile([128, NT], BF16)

        xT_d = xT.rearrange("(k p) n -> k p n", p=128)
        wq_d = wq.rearrange("(k p) c -> k p c", p=128)
        wk_d = wk.rearrange("(k p) c -> k p c", p=128)
        wv_d = wv.rearrange("(k p) c -> k p c", p=128)
        for k in range(8):
            nc.sync.dma_start(out=xT_sb[:, k, :], in_=xT_d[k])
            nc.sync.dma_start(out=wq_sb[:, k, :], in_=wq_d[k])
            nc.sync.dma_start(out=wk_sb[:, k, :], in_=wk_d[k])
            nc.sync.dma_start(out=wv_sb[:, k, :], in_=wv_d[k])
        nc.sync.dma_start(out=bq_sb, in_=bq)
        nc.sync.dma_start(out=bk_sb, in_=bk)
        nc.sync.dma_start(out=bv_sb, in_=bv)
        nc.sync.dma_start(out=wo_sb, in_=wo)

        # ---- emit helpers (psum shared with the scores tag) ----
        def emit_v_tile(tt):
            # V_aug [t,130] += xT[k, t-tile].T @ wv[k]
            psv = psp.tile([128, 130], F32, tag="ps")
            for k in range(8):
                nc.tensor.matmul(psv, lhsT=xT_sb[:, k, ts(tt, 128)],
                                 rhs=wv_sb[:, k, :],
                                 start=(k == 0), stop=(k == 7))
            nc.vector.tensor_add(V_sb[:, tt, :], psv, bv_sb)

        def emit_wo_tile(tt):
            for eh in range(2):
                pw = psp.tile([128, 512], F32, tag="ps")
                nc.tensor.matmul(pw, lhsT=attnT[:, ts(tt, 128)],
                                 rhs=wo_sb[:, ts(eh, 512)],
                                 start=True, stop=True)
                ob = stg.tile([128, 512], F32, tag="ob")
                nc.vector.tensor_copy(ob, pw)
                nc.sync.dma_start(
                    out=out[tt * 128:(tt + 1) * 128, eh * 512:(eh + 1) * 512],
                    in_=ob)

        def emit_proj_chunk(w_sb, b_sb, o_sb, n):
            # 512-token chunk n of the Q^T or K^T projection
            ps = psp.tile([128, 512], F32, tag="ps")
            for k in range(8):
                nc.tensor.matmul(ps, lhsT=w_sb[:, k, :],
                                 rhs=xT_sb[:, k, ts(n, 512)],
                                 start=(k == 0), stop=(k == 7))
            nc.vector.tensor_scalar_add(o_sb[:, ts(n, 512)], ps, b_sb)

        def emit_pv_step(prev, tt):
            # 4 PV matmuls for t-tile tt of the previous chunk
            b, sc, pts, psos = prev
            for h in range(HPC):
                for n2 in range(SC // 512):
                    nc.tensor.matmul(
                        psos[h][0:65, ts(n2, 512)],
                        lhsT=V_sb[:, b * 16 + tt, h * 65:(h + 1) * 65],
                        rhs=pts[tt][h][:, ts(n2, 512)],
                        start=(tt == 0), stop=(tt == 15))

        def emit_normalize(prev):
            b, sc, pts, psos = prev
            s0 = b * S + sc * SC
            for h in range(HPC):
                pso = psos[h]
                rsum = nrm.tile([1, SC], F32, tag="rsum")
                nc.vector.tensor_copy(rsum, pso[64:65, :])
                nc.vector.reciprocal_approx_fast(out=rsum, in_=rsum)
                recb = nrm.tile([64, SC], F32, tag="recb")
                nc.gpsimd.partition_broadcast(recb, rsum)
                nc.vector.tensor_mul(
                    attnT[h * DK:(h + 1) * DK, s0:s0 + SC],
                    pso[0:64, :], recb)

        # ---- minimal prologue: just enough of Q^T/K^T for chunk 0 ----
        emit_proj_chunk(wk_sb, bk_sb, KT, 0)
        for n in (0, 1):
            emit_proj_chunk(wq_sb, bq_sb, QT, n)

        # deferred PE work, interleaved into the ACT-bound attention loop.
        # entry = (step, thunk): emitted at the given tt step of that chunk.
        qk = [(wq_sb, bq_sb, QT), (wk_sb, bk_sb, KT)]
        extras_per_chunk = [
            # chunk 0 (b0,sc0): KT 1-3 just ahead of use, QT 2-3 (for sc1),
            # V tiles 0-15 (b0, needed by chunk 1)
            [(1, lambda n=n: emit_proj_chunk(*qk[1], n)) for n in (1, 2, 3)]
            + [(6, lambda n=n: emit_proj_chunk(*qk[0], n)) for n in (2, 3)]
            + [(i, lambda t=t: emit_v_tile(t)) for i, t in enumerate(range(16))],
            # chunk 1 (b0,sc1): QT 4-5, KT 4, V 16-31 (b1)
            [(1, lambda: emit_proj_chunk(*qk[0], 4)),
             (3, lambda: emit_proj_chunk(*qk[0], 5)),
             (5, lambda: emit_proj_chunk(*qk[1], 4))]
            + [(i, lambda t=t: emit_v_tile(t)) for i, t in enumerate(range(16, 32))],
            # chunk 2 (b1,sc0): KT 5-7 ahead of use, QT 6-7, WO tiles 0-7
            [(1, lambda: emit_proj_chunk(*qk[1], 5)),
             (5, lambda: emit_proj_chunk(*qk[1], 6)),
             (9, lambda: emit_proj_chunk(*qk[1], 7)),
             (3, lambda: emit_proj_chunk(*qk[0], 6)),
             (7, lambda: emit_proj_chunk(*qk[0], 7))]
            + [(2 * i, lambda t=t: emit_wo_tile(t)) for i, t in enumerate(range(8))],
            # chunk 3 (b1,sc1): WO tiles 8-15
            [(2 * i, lambda t=t: emit_wo_tile(t)) for i, t in enumerate(range(8, 16))],
        ]

        chunks = [(b, sc) for b in range(B) for sc in range(S // SC)]
        prev = None
        for ci, (b, sc) in enumerate(chunks):
            s0 = b * S + sc * SC
            extras = sorted(extras_per_chunk[ci], key=lambda e: e[0])
            pts = []
            psos = [pvp.tile([128, SC], F32, tag="pv", name=f"pso{ci}_{h}")
                    for h in range(HPC)]
            cur = (b, sc, pts, psos)
            for tt in range(16):
                row = []
                for h in range(HPC):
                    ps = psp.tile([128, SC], F32, tag="ps")
                    hsl = slice(h * DK, (h + 1) * DK)
                    for n2 in range(SC // 512):
                        nc.tensor.matmul(
                            ps[:, ts(n2, 512)],
                            lhsT=KT[hsl, b * S + tt * 128:b * S + (tt + 1) * 128],
                            rhs=QT[hsl, s0 + n2 * 512:s0 + (n2 + 1) * 512],
                            start=True, stop=True)
                    pt = ptp.tile([128, SC], BF16, tag="pt")
                    nc.scalar.activation(
                        out=pt, in_=ps,
                        func=mybir.ActivationFunctionType.Exp,
                        scale=0.125)
                    row.append(pt)
                pts.append(row)
                if prev is not None:
                    emit_pv_step(prev, tt)
                while extras and extras[0][0] <= tt:
                    extras.pop(0)[1]()
            for _, e in extras:
                e()
            if prev is not None:
                emit_normalize(prev)
            prev = cur
        # tail: PV + normalize for the last chunk, then remaining WO tiles
        for tt in range(16):
            emit_pv_step(prev, tt)
        emit_normalize(prev)
        for tt in range(16, 32):
            emit_wo_tile(tt)


def _prep_in_maps(x, wq, bq, wk, bk, wv, bv, wo):
    x2 = np.asarray(x, np.float32).reshape(NT, D)
    xT = np.ascontiguousarray(x2.T).astype(NPBF16)
    wq = np.asarray(wq, np.float32)
    wk = np.asarray(wk, np.float32)
    wv = np.asarray(wv, np.float32)
    wo = np.asarray(wo, np.float32)
    bq = np.asarray(bq, np.float32)
    bk = np.asarray(bk, np.float32)
    bv = np.asarray(bv, np.float32)
    in_maps = []
    for c in range(NCORES):
        cs = slice(c * 128, (c + 1) * 128)
        wv_aug = np.zeros((D, 130), np.float32)
        wv_aug[:, 0:64] = wv[:, c * 128:c * 128 + 64]
        wv_aug[:, 65:129] = wv[:, c * 128 + 64:c * 128 + 128]
        bv_aug = np.zeros(130, np.float32)
        bv_aug[0:64] = bv[c * 128:c * 128 + 64]
        bv_aug[64] = 1.0
        bv_aug[65:129] = bv[c * 128 + 64:c * 128 + 128]
        bv_aug[129] = 1.0
        in_maps.append({
            "xT": xT,
            "wq": wq[:, cs].astype(NPBF16),
            "wk": wk[:, cs].astype(NPBF16),
            "wv": wv_aug.astype(NPBF16),
            "bq": np.ascontiguousarray(bq[cs].reshape(128, 1)),
            "bk": np.ascontiguousarray(bk[cs].reshape(128, 1)),
            "bv": np.ascontiguousarray(np.broadcast_to(bv_aug, (128, 130))),
            "wo": wo[cs, :].astype(NPBF16),
        })
    return in_maps


def kernel(x, wq, bq, wk, bk, wv, bv, wo, bo, _run_kwargs=None):
    if "nc" not in _CACHE:
        _CACHE["nc"] = _build_nc()
    nc = _CACHE["nc"]
    in_maps = _prep_in_maps(x, wq, bq, wk, bk, wv, bv, wo)
    res = run_bass_kernel_spmd(nc, in_maps, list(range(NCORES)),
                               **(_run_kwargs or {}))
    acc = np.zeros((NT, D), np.float32)
    for c in range(NCORES):
        acc += res.results[c]["out"]
    acc += np.asarray(bo, np.float32)[None, :]
    if _run_kwargs:
        _CACHE["last_results"] = res
    return acc.reshape(B, S, D)


# revision 17
# speedup vs baseline: 1.2489x; 1.0022x over previous
"""Multi-head self-attention on 8 Trainium2 NeuronCores.

Tensor-parallel over heads: core c owns heads 2c, 2c+1 (128 of the 1024
hidden columns).  Each core:
  1. Qt/Kt = (x @ wq/wk + b)^T  in [d, token] layout (d on partitions,
     2 heads stacked: partitions 0:64 head0, 64:128 head1)
  2. V_aug = x @ [wv | 0] + [bv | 1]  in [token, 65-per-head] layout
     (ones column -> softmax denominator comes out of the P@V matmul)
  3. per (batch, head): S^T tiles = K^T.T @ Q^T  (contraction d=64),
     P^T = exp(S^T / 8) (no max subtraction needed: |S/8| < ~3),
     out^T[65, s] = V_aug.T @ P^T accumulated over t tiles,
     attnT = out^T[0:64] * broadcast(1 / out^T[64])
  4. partial = attnT.T @ wo[128 rows of this core]  -> HBM (f32)
Host sums the 8 partials and adds bo.

Shapes hardcoded for x:[2,2048,1024], 16 heads, d_k=64.
"""

import numpy as np
import ml_dtypes

import concourse.bass as bass
import concourse.tile as tile
from concourse import bacc, mybir
from concourse.bass import ts
from concourse.bass_utils import run_bass_kernel_spmd

BF16 = mybir.dt.bfloat16
F32 = mybir.dt.float32
NPBF16 = ml_dtypes.bfloat16

B = 2
S = 2048
D = 1024
NT = B * S  # 4096 tokens
DK = 64
NCORES = 8
HPC = 2  # heads per core
SC = 1024  # attention s-chunk (exp op free size)

_CACHE = {}


def _build_nc():
    nc = bacc.Bacc("TRN2", target_bir_lowering=False, debug=False,
                   num_devices=NCORES)

    xT = nc.dram_tensor("xT", [D, NT], BF16, kind="ExternalInput").ap()
    wq = nc.dram_tensor("wq", [D, 128], BF16, kind="ExternalInput").ap()
    wk = nc.dram_tensor("wk", [D, 128], BF16, kind="ExternalInput").ap()
    wv = nc.dram_tensor("wv", [D, 130], BF16, kind="ExternalInput").ap()
    bq = nc.dram_tensor("bq", [128, 1], F32, kind="ExternalInput").ap()
    bk = nc.dram_tensor("bk", [128, 1], F32, kind="ExternalInput").ap()
    bv = nc.dram_tensor("bv", [128, 130], F32, kind="ExternalInput").ap()
    wo = nc.dram_tensor("wo", [128, D], BF16, kind="ExternalInput").ap()
    out = nc.dram_tensor("out", [NT, D], F32, kind="ExternalOutput").ap()

    with tile.TileContext(nc) as tc:
        _emit(nc, tc, xT, wq, wk, wv, bq, bk, bv, wo, out)
    nc.compile()
    return nc


def _emit(nc, tc, xT, wq, wk, wv, bq, bk, bv, wo, out):
    import contextlib
    ctx = contextlib.ExitStack()
    with ctx:
        consts = ctx.enter_context(tc.tile_pool(name="consts", bufs=1))
        ptp = ctx.enter_context(tc.tile_pool(name="ptp", bufs=36))
        psp = ctx.enter_context(tc.tile_pool(name="psp", bufs=2, space="PSUM"))
        pvp = ctx.enter_context(tc.tile_pool(name="pvp", bufs=2, space="PSUM"))
        stg = ctx.enter_context(tc.tile_pool(name="stg", bufs=4))
        nrm = ctx.enter_context(tc.tile_pool(name="nrm", bufs=2))

        # ---- persistent SBUF tensors ----
        xT_sb = consts.tile([128, 8, NT], BF16)      # 8 k-tiles of x^T
        wq_sb = consts.tile([128, 8, 128], BF16)
        wk_sb = consts.tile([128, 8, 128], BF16)
        wv_sb = consts.tile([128, 8, 130], BF16)
        bq_sb = consts.tile([128, 1], F32)
        bk_sb = consts.tile([128, 1], F32)
        bv_sb = consts.tile([128, 130], F32)
        wo_sb = consts.tile([128, D], BF16)
        QT = consts.tile([128, NT], BF16)
        KT = consts.tile([128, NT], BF16)
        V_sb = consts.tile([128, 32, 130], BF16)     # [t-in-tile, t_tile, col]
        attnT = consts.tile([128, NT], BF16)

        xT_d = xT.rearrange("(k p) n -> k p n", p=128)
        wq_d = wq.rearrange("(k p) c -> k p c", p=128)
        wk_d = wk.rearrange("(k p) c -> k p c", p=128)
        wv_d = wv.rearrange("(k p) c -> k p c", p=128)
        # weights + batch-0 half of x first, so compute starts ASAP
        nc.sync.dma_start(out=bq_sb, in_=bq)
        nc.sync.dma_start(out=bk_sb, in_=bk)
        nc.sync.dma_start(out=bv_sb, in_=bv)
        for k in range(8):
            nc.sync.dma_start(out=wq_sb[:, k, :], in_=wq_d[k])
            nc.sync.dma_start(out=wk_sb[:, k, :], in_=wk_d[k])
            nc.sync.dma_start(out=wv_sb[:, k, :], in_=wv_d[k])
        for k in range(8):
            nc.sync.dma_start(out=xT_sb[:, k, 0:S], in_=xT_d[k][:, 0:S])
        for k in range(8):
            nc.sync.dma_start(out=xT_sb[:, k, S:NT], in_=xT_d[k][:, S:NT])
        nc.sync.dma_start(out=wo_sb, in_=wo)

        # ---- emit helpers (psum shared with the scores tag) ----
        def emit_v_tile(tt):
            # V_aug [t,130] += xT[k, t-tile].T @ wv[k]
            psv = psp.tile([128, 130], F32, tag="ps")
            for k in range(8):
                nc.tensor.matmul(psv, lhsT=xT_sb[:, k, ts(tt, 128)],
                                 rhs=wv_sb[:, k, :],
                                 start=(k == 0), stop=(k == 7))
            nc.vector.tensor_add(V_sb[:, tt, :], psv, bv_sb)

        def emit_wo_tile(tt, use_act=False):
            for eh in range(2):
                pw = psp.tile([128, 512], F32, tag="ps")
                nc.tensor.matmul(pw, lhsT=attnT[:, ts(tt, 128)],
                                 rhs=wo_sb[:, ts(eh, 512)],
                                 start=True, stop=True)
                ob = stg.tile([128, 512], F32, tag="ob")
                if use_act and eh == 1:
                    nc.scalar.activation(
                        out=ob, in_=pw,
                        func=mybir.ActivationFunctionType.Copy, bias=0.0)
                else:
                    nc.vector.tensor_copy(ob, pw)
                nc.sync.dma_start(
                    out=out[tt * 128:(tt + 1) * 128, eh * 512:(eh + 1) * 512],
                    in_=ob)

        def emit_proj_chunk(w_sb, b_sb, o_sb, n, w=512):
            # w-token chunk n (units of w) of the Q^T or K^T projection
            ps = psp.tile([128, 512], F32, tag="ps")
            for k in range(8):
                nc.tensor.matmul(ps[:, 0:w], lhsT=w_sb[:, k, :],
                                 rhs=xT_sb[:, k, ts(n, w)],
                                 start=(k == 0), stop=(k == 7))
            nc.vector.tensor_scalar_add(o_sb[:, ts(n, w)], ps[:, 0:w], b_sb)

        def emit_pv_step(prev, tt):
            # 4 PV matmuls for t-tile tt of the previous chunk
            b, sc, pts, psos = prev
            for h in range(HPC):
                for n2 in range(SC // 512):
                    nc.tensor.matmul(
                        psos[h][0:65, ts(n2, 512)],
                        lhsT=V_sb[:, b * 16 + tt, h * 65:(h + 1) * 65],
                        rhs=pts[tt][h][:, ts(n2, 512)],
                        start=(tt == 0), stop=(tt == 15))

        def emit_normalize(prev):
            b, sc, pts, psos = prev
            s0 = b * S + sc * SC
            for h in range(HPC):
                pso = psos[h]
                rsum = nrm.tile([1, SC], F32, tag="rsum")
                nc.vector.tensor_copy(rsum, pso[64:65, :])
                nc.vector.reciprocal_approx_fast(out=rsum, in_=rsum)
                recb = nrm.tile([64, SC], F32, tag="recb")
                nc.gpsimd.partition_broadcast(recb, rsum)
                nc.vector.tensor_mul(
                    attnT[h * DK:(h + 1) * DK, s0:s0 + SC],
                    pso[0:64, :], recb)

        # ---- minimal prologue: just enough of Q^T/K^T for chunk 0 ----
        emit_proj_chunk(wk_sb, bk_sb, KT, 0)
        for n in (0, 1):
            emit_proj_chunk(wq_sb, bq_sb, QT, n)

        # deferred PE work, interleaved into the ACT-bound attention loop.
        # entry = (step, thunk): emitted at the given tt step of that chunk.
        # Q^T/K^T chunks are emitted at 256-wide granularity (n in units of
        # 256) so no single extra hogs the PE long enough to starve ScalarE.
        qk = [(wq_sb, bq_sb, QT), (wk_sb, bk_sb, KT)]

        def pj(which, n256):
            return lambda: emit_proj_chunk(*qk[which], n256, w=256)

        def spread(thunks, start, stop):
            # distribute thunks evenly over tt steps [start, stop)
            n = len(thunks)
            return [(start + (i * (stop - start)) // n, t)
                    for i, t in enumerate(thunks)]

        extras_per_chunk = [
            # chunk 0 (b0,sc0): KT 1-3 just ahead of use, QT 2-3 (for sc1),
            # V tiles 0-15 (b0, needed by chunk 1)
            spread([pj(1, n) for n in (2, 3)], 0, 3)
            + spread([pj(1, n) for n in (4, 5, 6, 7)], 3, 11)
            + spread([pj(0, n) for n in (4, 5, 6, 7)], 8, 16)
            + [(i, lambda t=t: emit_v_tile(t)) for i, t in enumerate(range(16))],
            # chunk 1 (b0,sc1): QT 4-5, KT 4, V 16-31 (b1)
            spread([pj(0, n) for n in (8, 9, 10, 11)], 0, 8)
            + spread([pj(1, n) for n in (8, 9)], 8, 12)
            + [(i, lambda t=t: emit_v_tile(t)) for i, t in enumerate(range(16, 32))],
            # chunk 2 (b1,sc0): KT 5-7 ahead of use, QT 6-7, WO tiles 0-7
            spread([pj(1, n) for n in (10, 11, 12, 13)], 0, 8)
            + spread([pj(1, n) for n in (14, 15)], 8, 12)
            + spread([pj(0, n) for n in (12, 13, 14, 15)], 4, 12)
            + [(2 * i, lambda t=t: emit_wo_tile(t)) for i, t in enumerate(range(8))],
            # chunk 3 (b1,sc1): WO tiles 8-15
            [(2 * i, lambda t=t: emit_wo_tile(t)) for i, t in enumerate(range(8, 16))],
        ]

        chunks = [(b, sc) for b in range(B) for sc in range(S // SC)]
        prev = None
        for ci, (b, sc) in enumerate(chunks):
            s0 = b * S + sc * SC
            extras = sorted(extras_per_chunk[ci], key=lambda e: e[0])
            pts = []
            psos = [pvp.tile([128, SC], F32, tag="pv", name=f"pso{ci}_{h}")
                    for h in range(HPC)]
            cur = (b, sc, pts, psos)
            for tt in range(16):
                row = []
                for h in range(HPC):
                    ps = psp.tile([128, SC], F32, tag="ps")
                    hsl = slice(h * DK, (h + 1) * DK)
                    for n2 in range(SC // 512):
                        nc.tensor.matmul(
                            ps[:, ts(n2, 512)],
                            lhsT=KT[hsl, b * S + tt * 128:b * S + (tt + 1) * 128],
                            rhs=QT[hsl, s0 + n2 * 512:s0 + (n2 + 1) * 512],
                            start=True, stop=True)
                    pt = ptp.tile([128, SC], BF16, tag="pt")
                    nc.scalar.activation(
                        out=pt, in_=ps,
                        func=mybir.ActivationFunctionType.Exp,
                        scale=0.125)
                    row.append(pt)
                pts.append(row)
                if prev is not None:
                    emit_pv_step(prev, tt)
                while extras and extras[0][0] <= tt:
                    extras.pop(0)[1]()
            for _, e in extras:
                e()
            if prev is not None:
                emit_normalize(prev)
            prev = cur
        # tail: PV + normalize for the last chunk, with WO 16-23 (already
        # normalized) interleaved; then the final WO tiles
        for tt in range(16):
            emit_pv_step(prev, tt)
            if tt % 2 == 1:
                emit_wo_tile(16 + tt // 2, use_act=True)
        emit_normalize(prev)
        for tt in range(24, 32):
            emit_wo_tile(tt, use_act=True)


def _prep_in_maps(x, wq, bq, wk, bk, wv, bv, wo):
    x2 = np.asarray(x, np.float32).reshape(NT, D)
    xT = np.ascontiguousarray(x2.T).astype(NPBF16)
    wq = np.asarray(wq, np.float32)
    wk = np.asarray(wk, np.float32)
    wv = np.asarray(wv, np.float32)
    wo = np.asarray(wo, np.float32)
    bq = np.asarray(bq, np.float32)
    bk = np.asarray(bk, np.float32)
    bv = np.asarray(bv, np.float32)
    in_maps = []
    for c in range(NCORES):
        cs = slice(c * 128, (c + 1) * 128)
        wv_aug = np.zeros((D, 130), np.float32)
        wv_aug[:, 0:64] = wv[:, c * 128:c * 128 + 64]
        wv_aug[:, 65:129] = wv[:, c * 128 + 64:c * 128 + 128]
        bv_aug = np.zeros(130, np.float32)
        bv_aug[0:64] = bv[c * 128:c * 128 + 64]
        bv_aug[64] = 1.0
        bv_aug[65:129] = bv[c * 128 + 64:c * 128 + 128]
        bv_aug[129] = 1.0
        in_maps.append({
            "xT": xT,
            "wq": wq[:, cs].astype(NPBF16),
            "wk": wk[:, cs].astype(NPBF16),
            "wv": wv_aug.astype(NPBF16),
            "bq": np.ascontiguousarray(bq[cs].reshape(128, 1)),
            "bk": np.ascontiguousarray(bk[cs].reshape(128, 1)),
            "bv": np.ascontiguousarray(np.broadcast_to(bv_aug, (128, 130))),
            "wo": wo[cs, :].astype(NPBF16),
        })
    return in_maps


def kernel(x, wq, bq, wk, bk, wv, bv, wo, bo, _run_kwargs=None):
    if "nc" not in _CACHE:
        _CACHE["nc"] = _build_nc()
    nc = _CACHE["nc"]
    in_maps = _prep_in_maps(x, wq, bq, wk, bk, wv, bv, wo)
    res = run_bass_kernel_spmd(nc, in_maps, list(range(NCORES)),
                               **(_run_kwargs or {}))
    acc = np.zeros((NT, D), np.float32)
    for c in range(NCORES):
        acc += res.results[c]["out"]
    acc += np.asarray(bo, np.float32)[None, :]
    if _run_kwargs:
        _CACHE["last_results"] = res
    return acc.reshape(B, S, D)


# revision 19
# speedup vs baseline: 1.2832x; 1.0274x over previous
"""Multi-head self-attention on 8 Trainium2 NeuronCores.

Tensor-parallel over heads: core c owns heads 2c, 2c+1 (128 of the 1024
hidden columns).  Each core:
  1. Qt/Kt = (x @ wq/wk + b)^T  in [d, token] layout (d on partitions,
     2 heads stacked: partitions 0:64 head0, 64:128 head1)
  2. V_aug = x @ [wv | 0] + [bv | 1]  in [token, 65-per-head] layout
     (ones column -> softmax denominator comes out of the P@V matmul)
  3. per (batch, head): S^T tiles = K^T.T @ Q^T  (contraction d=64),
     P^T = exp(S^T / 8) (no max subtraction needed: |S/8| < ~3),
     out^T[65, s] = V_aug.T @ P^T accumulated over t tiles,
     attnT = out^T[0:64] * broadcast(1 / out^T[64])
  4. partial = attnT.T @ wo[128 rows of this core]  -> HBM (f32)
Host sums the 8 partials and adds bo.

Shapes hardcoded for x:[2,2048,1024], 16 heads, d_k=64.
"""

import numpy as np
import ml_dtypes

import concourse.bass as bass
import concourse.tile as tile
from concourse import bacc, mybir
from concourse.bass import ts
from concourse.bass_utils import run_bass_kernel_spmd

BF16 = mybir.dt.bfloat16
F32 = mybir.dt.float32
NPBF16 = ml_dtypes.bfloat16

B = 2
S = 2048
D = 1024
NT = B * S  # 4096 tokens
DK = 64
NCORES = 8
HPC = 2  # heads per core
SC = 1024  # attention s-chunk (exp op free size)

_CACHE = {}


def _build_nc():
    nc = bacc.Bacc("TRN2", target_bir_lowering=False, debug=False,
                   num_devices=NCORES)

    xT = nc.dram_tensor("xT", [D, NT], BF16, kind="ExternalInput").ap()
    wq = nc.dram_tensor("wq", [D, 128], BF16, kind="ExternalInput").ap()
    wk = nc.dram_tensor("wk", [D, 128], BF16, kind="ExternalInput").ap()
    wv = nc.dram_tensor("wv", [D, 130], BF16, kind="ExternalInput").ap()
    bq = nc.dram_tensor("bq", [128, 1], F32, kind="ExternalInput").ap()
    bk = nc.dram_tensor("bk", [128, 1], F32, kind="ExternalInput").ap()
    bv = nc.dram_tensor("bv", [128, 130], F32, kind="ExternalInput").ap()
    wo = nc.dram_tensor("wo", [128, D], BF16, kind="ExternalInput").ap()
    out = nc.dram_tensor("out", [NT, D], F32, kind="ExternalOutput").ap()

    with tile.TileContext(nc) as tc:
        _emit(nc, tc, xT, wq, wk, wv, bq, bk, bv, wo, out)
    nc.compile()
    return nc


def _emit(nc, tc, xT, wq, wk, wv, bq, bk, bv, wo, out):
    import contextlib
    ctx = contextlib.ExitStack()
    with ctx:
        consts = ctx.enter_context(tc.tile_pool(name="consts", bufs=1))
        ptp = ctx.enter_context(tc.tile_pool(name="ptp", bufs=36))
        psp = ctx.enter_context(tc.tile_pool(name="psp", bufs=2, space="PSUM"))
        pvp = ctx.enter_context(tc.tile_pool(name="pvp", bufs=2, space="PSUM"))
        stg = ctx.enter_context(tc.tile_pool(name="stg", bufs=4))
        nrm = ctx.enter_context(tc.tile_pool(name="nrm", bufs=2))

        # ---- persistent SBUF tensors ----
        xT_sb = consts.tile([128, 8, NT], BF16)      # 8 k-tiles of x^T
        wq_sb = consts.tile([128, 8, 128], BF16)
        wk_sb = consts.tile([128, 8, 128], BF16)
        wv_sb = consts.tile([128, 8, 130], BF16)
        bq_sb = consts.tile([128, 1], F32)
        bk_sb = consts.tile([128, 1], F32)
        bv_sb = consts.tile([128, 130], F32)
        wo_sb = consts.tile([128, D], BF16)
        QT = consts.tile([128, NT], BF16)
        KT = consts.tile([128, NT], BF16)
        V_sb = consts.tile([128, 32, 130], BF16)     # [t-in-tile, t_tile, col]
        attnT = consts.tile([128, NT], BF16)

        xT_d = xT.rearrange("(k p) n -> k p n", p=128)
        # batch-0 half of x on the sync queue; weights (one consolidated DMA
        # each) + biases on the gpsimd queue in parallel — compute can start
        # as soon as x half 0 + wk land.
        for k in range(8):
            nc.sync.dma_start(out=xT_sb[:, k, 0:S], in_=xT_d[k][:, 0:S])
        nc.gpsimd.dma_start(out=bq_sb, in_=bq)
        nc.gpsimd.dma_start(out=bk_sb, in_=bk)
        nc.gpsimd.dma_start(out=bv_sb, in_=bv)
        nc.gpsimd.dma_start(out=wk_sb, in_=wk.rearrange("(k p) c -> p k c", p=128))
        nc.gpsimd.dma_start(out=wq_sb, in_=wq.rearrange("(k p) c -> p k c", p=128))
        nc.gpsimd.dma_start(out=wv_sb, in_=wv.rearrange("(k p) c -> p k c", p=128))
        nc.gpsimd.dma_start(out=wo_sb, in_=wo)
        for k in range(8):
            nc.sync.dma_start(out=xT_sb[:, k, S:NT], in_=xT_d[k][:, S:NT])

        # ---- emit helpers (psum shared with the scores tag) ----
        def emit_v_tile(tt):
            # V_aug [t,130] += xT[k, t-tile].T @ wv[k]
            psv = psp.tile([128, 130], F32, tag="ps")
            for k in range(8):
                nc.tensor.matmul(psv, lhsT=xT_sb[:, k, ts(tt, 128)],
                                 rhs=wv_sb[:, k, :],
                                 start=(k == 0), stop=(k == 7))
            nc.vector.tensor_add(V_sb[:, tt, :], psv, bv_sb)

        def emit_wo_tile(tt, use_act=False):
            for eh in range(2):
                pw = psp.tile([128, 512], F32, tag="ps")
                nc.tensor.matmul(pw, lhsT=attnT[:, ts(tt, 128)],
                                 rhs=wo_sb[:, ts(eh, 512)],
                                 start=True, stop=True)
                ob = stg.tile([128, 512], F32, tag="ob")
                if use_act and eh == 1:
                    nc.scalar.activation(
                        out=ob, in_=pw,
                        func=mybir.ActivationFunctionType.Copy, bias=0.0)
                else:
                    nc.vector.tensor_copy(ob, pw)
                nc.sync.dma_start(
                    out=out[tt * 128:(tt + 1) * 128, eh * 512:(eh + 1) * 512],
                    in_=ob)

        def emit_proj_chunk(w_sb, b_sb, o_sb, n, w=512):
            # w-token chunk n (units of w) of the Q^T or K^T projection
            ps = psp.tile([128, 512], F32, tag="ps")
            for k in range(8):
                nc.tensor.matmul(ps[:, 0:w], lhsT=w_sb[:, k, :],
                                 rhs=xT_sb[:, k, ts(n, w)],
                                 start=(k == 0), stop=(k == 7))
            nc.vector.tensor_scalar_add(o_sb[:, ts(n, w)], ps[:, 0:w], b_sb)

        def emit_pv_step(prev, tt):
            # 4 PV matmuls for t-tile tt of the previous chunk
            b, sc, pts, psos = prev
            for h in range(HPC):
                for n2 in range(SC // 512):
                    nc.tensor.matmul(
                        psos[h][0:65, ts(n2, 512)],
                        lhsT=V_sb[:, b * 16 + tt, h * 65:(h + 1) * 65],
                        rhs=pts[tt][h][:, ts(n2, 512)],
                        start=(tt == 0), stop=(tt == 15))

        def emit_normalize(prev):
            b, sc, pts, psos = prev
            s0 = b * S + sc * SC
            for h in range(HPC):
                pso = psos[h]
                rsum = nrm.tile([1, SC], F32, tag="rsum")
                nc.vector.tensor_copy(rsum, pso[64:65, :])
                nc.vector.reciprocal_approx_fast(out=rsum, in_=rsum)
                recb = nrm.tile([64, SC], F32, tag="recb")
                nc.gpsimd.partition_broadcast(recb, rsum)
                nc.vector.tensor_mul(
                    attnT[h * DK:(h + 1) * DK, s0:s0 + SC],
                    pso[0:64, :], recb)

        # ---- minimal prologue: just enough of Q^T/K^T for chunk 0 ----
        emit_proj_chunk(wk_sb, bk_sb, KT, 0)
        for n in (0, 1):
            emit_proj_chunk(wq_sb, bq_sb, QT, n)

        # deferred PE work, interleaved into the ACT-bound attention loop.
        # entry = (step, thunk): emitted at the given tt step of that chunk.
        # Q^T/K^T chunks are emitted at 256-wide granularity (n in units of
        # 256) so no single extra hogs the PE long enough to starve ScalarE.
        qk = [(wq_sb, bq_sb, QT), (wk_sb, bk_sb, KT)]

        def pj(which, n256):
            return lambda: emit_proj_chunk(*qk[which], n256, w=256)

        def spread(thunks, start, stop):
            # distribute thunks evenly over tt steps [start, stop)
            n = len(thunks)
            return [(start + (i * (stop - start)) // n, t)
                    for i, t in enumerate(thunks)]

        extras_per_chunk = [
            # chunk 0 (b0,sc0): KT 1-3 just ahead of use, QT 2-3 (for sc1),
            # V tiles 0-15 (b0, needed by chunk 1)
            spread([pj(1, n) for n in (2, 3)], 0, 3)
            + spread([pj(1, n) for n in (4, 5, 6, 7)], 3, 11)
            + spread([pj(0, n) for n in (4, 5, 6, 7)], 8, 16)
            + [(i, lambda t=t: emit_v_tile(t)) for i, t in enumerate(range(16))],
            # chunk 1 (b0,sc1): QT 4-5, KT 4, V 16-31 (b1)
            spread([pj(0, n) for n in (8, 9, 10, 11)], 0, 8)
            + spread([pj(1, n) for n in (8, 9)], 8, 12)
            + [(i, lambda t=t: emit_v_tile(t)) for i, t in enumerate(range(16, 32))],
            # chunk 2 (b1,sc0): KT 5-7 ahead of use, QT 6-7, WO tiles 0-7
            spread([pj(1, n) for n in (10, 11, 12, 13)], 0, 8)
            + spread([pj(1, n) for n in (14, 15)], 8, 12)
            + spread([pj(0, n) for n in (12, 13, 14, 15)], 4, 12)
            + [(2 * i, lambda t=t: emit_wo_tile(t)) for i, t in enumerate(range(8))],
            # chunk 3 (b1,sc1): WO tiles 8-15
            [(2 * i, lambda t=t: emit_wo_tile(t)) for i, t in enumerate(range(8, 16))],
        ]

        def emit_scores(b, sc, tt):
            # one t-tile of S^T for both heads -> psum pair; returns the pair
            s0 = b * S + sc * SC
            pair = []
            for h in range(HPC):
                ps = psp.tile([128, SC], F32, tag="ps")
                hsl = slice(h * DK, (h + 1) * DK)
                for n2 in range(SC // 512):
                    nc.tensor.matmul(
                        ps[:, ts(n2, 512)],
                        lhsT=KT[hsl, b * S + tt * 128:b * S + (tt + 1) * 128],
                        rhs=QT[hsl, s0 + n2 * 512:s0 + (n2 + 1) * 512],
                        start=True, stop=True)
                pair.append(ps)
            return pair

        chunks = [(b, sc) for b in range(B) for sc in range(S // SC)]
        prev = None
        for ci, (b, sc) in enumerate(chunks):
            extras = sorted(extras_per_chunk[ci], key=lambda e: e[0])
            pts = []
            psos = [pvp.tile([128, SC], F32, tag="pv", name=f"pso{ci}_{h}")
                    for h in range(HPC)]
            cur = (b, sc, pts, psos)
            pair = emit_scores(b, sc, 0)
            for tt in range(16):
                row = []
                for h in range(HPC):
                    pt = ptp.tile([128, SC], BF16, tag="pt")
                    nc.scalar.activation(
                        out=pt, in_=pair[h],
                        func=mybir.ActivationFunctionType.Exp,
                        scale=0.125)
                    row.append(pt)
                pts.append(row)
                # emit next scores ahead of the slower PE work so ScalarE's
                # psum slots refill as soon as its exp frees them
                if tt + 1 < 16:
                    pair = emit_scores(b, sc, tt + 1)
                if prev is not None:
                    emit_pv_step(prev, tt)
                while extras and extras[0][0] <= tt:
                    extras.pop(0)[1]()
            for _, e in extras:
                e()
            if prev is not None:
                emit_normalize(prev)
            prev = cur
        # tail: PV + normalize for the last chunk, with WO 16-23 (already
        # normalized) interleaved; then the final WO tiles
        for tt in range(16):
            emit_pv_step(prev, tt)
            if tt % 2 == 1:
                emit_wo_tile(16 + tt // 2, use_act=True)
        emit_normalize(prev)
        for tt in range(24, 32):
            emit_wo_tile(tt, use_act=True)


def _prep_in_maps(x, wq, bq, wk, bk, wv, bv, wo):
    x2 = np.asarray(x, np.float32).reshape(NT, D)
    xT = np.ascontiguousarray(x2.T).astype(NPBF16)
    wq = np.asarray(wq, np.float32)
    wk = np.asarray(wk, np.float32)
    wv = np.asarray(wv, np.float32)
    wo = np.asarray(wo, np.float32)
    bq = np.asarray(bq, np.float32)
    bk = np.asarray(bk, np.float32)
    bv = np.asarray(bv, np.float32)
    in_maps = []
    for c in range(NCORES):
        cs = slice(c * 128, (c + 1) * 128)
        wv_aug = np.zeros((D, 130), np.float32)
        wv_aug[:, 0:64] = wv[:, c * 128:c * 128 + 64]
        wv_aug[:, 65:129] = wv[:, c * 128 + 64:c * 128 + 128]
        bv_aug = np.zeros(130, np.float32)
        bv_aug[0:64] = bv[c * 128:c * 128 + 64]
        bv_aug[64] = 1.0
        bv_aug[65:129] = bv[c * 128 + 64:c * 128 + 128]
        bv_aug[129] = 1.0
        in_maps.append({
            "xT": xT,
            "wq": wq[:, cs].astype(NPBF16),
            "wk": wk[:, cs].astype(NPBF16),
            "wv": wv_aug.astype(NPBF16),
            "bq": np.ascontiguousarray(bq[cs].reshape(128, 1)),
            "bk": np.ascontiguousarray(bk[cs].reshape(128, 1)),
            "bv": np.ascontiguousarray(np.broadcast_to(bv_aug, (128, 130))),
            "wo": wo[cs, :].astype(NPBF16),
        })
    return in_maps


def kernel(x, wq, bq, wk, bk, wv, bv, wo, bo, _run_kwargs=None):
    if "nc" not in _CACHE:
        _CACHE["nc"] = _build_nc()
    nc = _CACHE["nc"]
    in_maps = _prep_in_maps(x, wq, bq, wk, bk, wv, bv, wo)
    res = run_bass_kernel_spmd(nc, in_maps, list(range(NCORES)),
                               **(_run_kwargs or {}))
    acc = np.zeros((NT, D), np.float32)
    for c in range(NCORES):
        acc += res.results[c]["out"]
    acc += np.asarray(bo, np.float32)[None, :]
    if _run_kwargs:
        _CACHE["last_results"] = res
    return acc.reshape(B, S, D)
